# revision 7
# baseline (speedup 1.0000x reference)
"""AttnBlock (GroupNorm + single-head full attention + residual) on 8 TRN2 cores.

Reference computation (B=4, C=256, L=4096, fp32):
    xn   = GroupNorm32(x) * gn_w + gn_b
    q, k, v = 1x1 convs of xn;  attn = softmax(q^T k / sqrt(C)) ; out = x + pw @ (attn v)

Sharding: 8 cores = 4 batches x 2 query-halves. Each core computes GroupNorm +
K/V over the full sequence of its batch element, and Q/attention for its half
of the queries (Lq = 2048). No collectives; the host slices inputs and
concatenates outputs.

Per-core kernel structure:
  - GroupNorm stats via bn_stats/bn_aggr per partition row, then cross-partition
    group reduction + broadcast-back with tiny indicator matmuls on the PE.
  - Q/K/V projections with float32r matmuls (full PE rate, ~1e-4 accuracy).
  - v is immediately re-projected through pw:  pvT[j, o] = (pw @ v)^T, with an
    extra ones-column appended.  Then attention output and softmax row-sums
    come from ONE fused matmul chain:
        finT[i, (o|sum)] = sum_j exp(sT)[j, i] * pvT[j, (o|1)]
  - Scores are computed transposed (sT[j, i]) so softmax reduction over keys j
    is the matmul contraction, never a cross-partition op.  Logits are in
    [-6.2, 6.0] (std ~1 by construction), so exp needs no max subtraction.
  - Normalize + residual in one DVE op: out = (finT * 1/sum) + (x^T + pb_eff),
    with x^T built once via PE transposes.  Output is [Lq, C] per core; the
    host transposes back.  vb/pb fold into pb_eff = pb + pw @ vb on the host.
"""

import numpy as np
from contextlib import ExitStack

import concourse.bass as bass
import concourse.tile as tile
from concourse import mybir
from concourse.bass_utils import run_bass_kernel_spmd
from concourse.vector_clock import ScopedClock
import bass_rust

F32 = mybir.dt.float32
F32R = mybir.dt.float32r
BF16 = mybir.dt.bfloat16
AF = mybir.ActivationFunctionType
OP = mybir.AluOpType

B, C, L = 4, 256, 4096
G = 32
EPS = 1e-6
NCORES = 8
LQ = L // 2  # queries per core
CT = C // 128  # 2 channel tiles
JT = L // 128  # 32 key tiles
NIB = 4  # i-blocks of 512 queries
IBS = 512
NIS = LQ // 128  # 16 query slices of 128


class TC(tile.TileContext):
    """This walrus build caps sync-waits per instruction at 1; Tile attaches
    several to one instruction.  Hoist extras onto same-engine NOPs."""

    def _drain_and_barrier(self, tick_clock, wait_clock):
        collector = self.nc.sync.nop(nofuse=True)
        wait_clock.add_sem_waits(
            collector.ins, ScopedClock({None: tick_clock.global_clock})
        )
        waits = (
            list(collector.ins.sync_info.on_wait)
            if collector.ins.sync_info is not None
            else []
        )
        collector.ins.sync_info = bass_rust.SyncInfo(on_wait=[], on_update=[])
        for w in waits:
            n2 = self.nc.sync.nop(nofuse=True)
            n2.ins.sync_info = bass_rust.SyncInfo(on_wait=[w], on_update=[])
        self.nc.sync.drain()
        self.nc.all_engine_barrier()
        assert self.sems is not None
        popped = self.nc._tile_sem_poison_stack.pop()
        assert popped is self._sem_poison
        self.nc.clear_and_free_semaphores(list(self.sems.allocated().values()))
        self.nc.all_engine_barrier()


def split_sync_waits(nc, max_waits=1):
    ctr = 0
    for fn in nc.m.functions:
        for bb in fn.blocks:
            old = list(bb.instructions)
            new = []
            changed = False
            for inst in old:
                si = inst.sync_info
                if si is not None and len(si.on_wait) > max_waits:
                    waits = list(si.on_wait)
                    extra, keep = waits[:-max_waits], waits[-max_waits:]
                    for i in range(0, len(extra), max_waits):
                        nop = mybir.InstNoOp(name=f"I-waitnop-{ctr}")
                        ctr += 1
                        nop.engine = inst.engine
                        nop.sync_info = bass_rust.SyncInfo(
                            on_wait=extra[i : i + max_waits], on_update=[]
                        )
                        new.append(nop)
                        changed = True
                    inst.sync_info = bass_rust.SyncInfo(
                        on_wait=keep, on_update=list(si.on_update)
                    )
                new.append(inst)
            if changed:
                bb.instructions = new


def _build_program():
    nc = bass.Bass()

    x_d = nc.declare_dram_parameter("x_full", [C, L], F32, isOutput=False)
    xc_d = nc.declare_dram_parameter("x_cols", [C, LQ], F32, isOutput=False)
    qwT_d = nc.declare_dram_parameter("qwT", [C, C], F32, isOutput=False)
    kwT_d = nc.declare_dram_parameter("kwT", [C, C], F32, isOutput=False)
    vwT_d = nc.declare_dram_parameter("vwT", [C, C], F32, isOutput=False)
    pwT_d = nc.declare_dram_parameter("pwT", [C, C], F32, isOutput=False)
    qb_d = nc.declare_dram_parameter("qb2", [C, 1], F32, isOutput=False)
    kb_d = nc.declare_dram_parameter("kb2", [C, 1], F32, isOutput=False)
    pbe_d = nc.declare_dram_parameter("pbe", [1, C], F32, isOutput=False)
    gnw_d = nc.declare_dram_parameter("gnw", [C, 1], F32, isOutput=False)
    gnb_d = nc.declare_dram_parameter("gnb", [C, 1], F32, isOutput=False)
    id_d = nc.declare_dram_parameter("ident", [128, 128], F32, isOutput=False)
    ind_d = nc.declare_dram_parameter("ind", [128, 2 * G], F32, isOutput=False)
    bc_d = nc.declare_dram_parameter("bc", [G, C], F32, isOutput=False)
    out_d = nc.declare_dram_parameter("out", [LQ, C], F32, isOutput=True)

    with TC(nc) as tc, ExitStack() as ctx:
        const = ctx.enter_context(tc.tile_pool(name="const", bufs=1))

        ident = const.tile([128, 128], F32, tag="ident")
        nc.sync.dma_start(out=ident[:], in_=id_d[:])
        ind_t = const.tile([128, 2, G], F32, tag="ind")
        nc.sync.dma_start(out=ind_t[:], in_=ind_d[:].rearrange("p (t g) -> p t g", t=2))
        bc_t = const.tile([G, 2, 128], F32, tag="bc")
        nc.sync.dma_start(out=bc_t[:], in_=bc_d[:].rearrange("g (t p) -> g t p", t=2))
        pbb = const.tile([128, C], F32, tag="pbb")
        nc.sync.dma_start(out=pbb[:], in_=pbe_d[:].to_broadcast([128, C]))

        def col2(tag, src):
            t = const.tile([128, 2, 1], F32, tag=tag)
            nc.sync.dma_start(
                out=t[:], in_=src[:].rearrange("(t p) o -> p t o", p=128)
            )
            return t

        gnw_t = col2("gnw", gnw_d)
        gnb_t = col2("gnb", gnb_d)
        qb_t = col2("qb", qb_d)
        kb_t = col2("kb", kb_d)

        qwT_t = const.tile([128, 2, C], F32R, tag="qwT")
        kwT_t = const.tile([128, 2, C], F32R, tag="kwT")
        vwT_t = const.tile([128, 2, C], F32R, tag="vwT")
        pwT_t = const.tile([128, 2, C], BF16, tag="pwT")
        with tc.tile_pool(name="wstage", bufs=2) as wst:
            for w_d, w_t in (
                (qwT_d, qwT_t),
                (kwT_d, kwT_t),
                (vwT_d, vwT_t),
                (pwT_d, pwT_t),
            ):
                st = wst.tile([128, 2, C], F32, tag="wst")
                nc.sync.dma_start(
                    out=st[:], in_=w_d[:].rearrange("(t p) o -> p t o", p=128)
                )
                nc.vector.tensor_copy(out=w_t[:], in_=st[:])

        xt_p = ctx.enter_context(tc.tile_pool(name="xt", bufs=1))
        outp = ctx.enter_context(tc.tile_pool(name="outp", bufs=4))
        qkv = ctx.enter_context(tc.tile_pool(name="qkv", bufs=1))
        pvt_p = ctx.enter_context(tc.tile_pool(name="pvt", bufs=1))
        small = ctx.enter_context(tc.tile_pool(name="small", bufs=1))
        rpool = ctx.enter_context(tc.tile_pool(name="rpool", bufs=4))

        xT = xt_p.tile([128, NIS, C], F32, tag="xT")
        q_t = qkv.tile([128, 2, LQ], F32R, tag="q")
        k_t = qkv.tile([128, 2, L], F32R, tag="k")
        v_t = qkv.tile([128, 2, L], BF16, tag="v")
        pvT = pvt_p.tile([128, JT, C + 1], BF16, tag="pvT")

        # ---------------- Phase A: GroupNorm, projections, pvT, xT ----------
        with (
            tc.tile_pool(name="xbuf", bufs=1) as xbuf,
            tc.tile_pool(name="xnp", bufs=3) as xnp,
            tc.tile_pool(name="psA", bufs=4, space="PSUM") as psA,
        ):
            xf = xbuf.tile([128, 2, L], F32, tag="xf")
            xc = xbuf.tile([128, 2, LQ], F32, tag="xc")
            for t in range(2):
                for h in range(2):
                    nc.sync.dma_start(
                        out=xf[:, t, h * 2048 : (h + 1) * 2048],
                        in_=x_d[:]
                        .rearrange("(t p) l -> p t l", p=128)[
                            :, t, h * 2048 : (h + 1) * 2048
                        ],
                    )
                nc.sync.dma_start(
                    out=xc[:, t, :],
                    in_=xc_d[:].rearrange("(t p) l -> p t l", p=128)[:, t, :],
                )

            # GroupNorm statistics
            stats = small.tile([128, 2, 8, 6], F32, tag="stats")
            mv = small.tile([128, 2, 2], F32, tag="mv")
            for t in range(2):
                xv = xf[:, t, :].rearrange("p (s f) -> p s f", f=512)
                for s in range(8):
                    nc.vector.bn_stats(out=stats[:, t, s, :], in_=xv[:, s, :])
                nc.vector.bn_aggr(out=mv[:, t, :], in_=stats[:, t, :, :])
                # var slot <- E[x^2] = m*m + var
                nc.vector.tensor_scalar(
                    out=mv[:, t, 1:2],
                    in0=mv[:, t, 0:1],
                    scalar1=mv[:, t, 0:1],
                    scalar2=mv[:, t, 1:2],
                    op0=OP.mult,
                    op1=OP.add,
                )
            psg = psA.tile([G, 2], F32, tag="mm")
            nc.tensor.matmul(
                out=psg[:], lhsT=ind_t[:, 0, :], rhs=mv[:, 0, :], start=True, stop=False
            )
            nc.tensor.matmul(
                out=psg[:], lhsT=ind_t[:, 1, :], rhs=mv[:, 1, :], start=False, stop=True
            )
            g2 = small.tile([G, 2], F32, tag="g2")  # [mu, rstd]
            nvar = small.tile([G, 1], F32, tag="nvar")
            sq = small.tile([G, 1], F32, tag="sq")
            eps_t = small.tile([G, 1], F32, tag="eps")
            nc.vector.memset(eps_t[:], float(EPS))
            nc.vector.tensor_scalar_mul(out=g2[:, 0:1], in0=psg[:, 0:1], scalar1=0.125)
            nc.vector.tensor_scalar_mul(out=g2[:, 1:2], in0=psg[:, 1:2], scalar1=0.125)
            nc.vector.tensor_scalar(
                out=nvar[:],
                in0=g2[:, 0:1],
                scalar1=g2[:, 0:1],
                scalar2=g2[:, 1:2],
                op0=OP.mult,
                op1=OP.subtract,
            )  # mu^2 - E[x^2] = -var
            nc.scalar.activation(
                out=sq[:], in_=nvar[:], func=AF.Sqrt, bias=eps_t[:], scale=-1.0
            )
            nc.vector.reciprocal(out=g2[:, 1:2], in_=sq[:])

            # broadcast group stats back to channels; per-channel scale/bias
            sca = small.tile([128, 2, 2], F32, tag="sca")  # [s, t] per channel tile
            mneg = small.tile([128, 1], F32, tag="mneg")
            for t in range(2):
                psb = psA.tile([128, 2], F32, tag="mm")
                nc.tensor.matmul(
                    out=psb[:], lhsT=bc_t[:, t, :], rhs=g2[:], start=True, stop=True
                )
                nc.vector.tensor_mul(
                    out=sca[:, t, 0:1], in0=psb[:, 1:2], in1=gnw_t[:, t, :]
                )
                nc.vector.tensor_scalar_mul(
                    out=mneg[:], in0=psb[:, 0:1], scalar1=-1.0
                )
                nc.vector.scalar_tensor_tensor(
                    out=sca[:, t, 1:2],
                    in0=mneg[:],
                    scalar=sca[:, t, 0:1],
                    in1=gnb_t[:, t, :],
                    op0=OP.mult,
                    op1=OP.add,
                )

            # x^T for the residual (+ pb_eff folded in), from original x_cols
            for isl in range(NIS):
                for t in range(2):
                    pst = psA.tile([128, 128], F32, tag="mm")
                    nc.tensor.transpose(
                        out=pst[:],
                        in_=xc[:, t, isl * 128 : (isl + 1) * 128],
                        identity=ident[:],
                    )
                    nc.vector.tensor_add(
                        out=xT[:, isl, t * 128 : (t + 1) * 128],
                        in0=pst[:],
                        in1=pbb[:, t * 128 : (t + 1) * 128],
                    )

            # GroupNorm apply (rounding to f32r) + K/V/Q projections, streamed
            # per 512-column chunk so the normalized activations never live in
            # full in SBUF.
            for ch in range(8):
                sl = slice(ch * 512, (ch + 1) * 512)
                xn_c = xnp.tile([128, 2, 512], F32R, tag="xn")
                for t in range(2):
                    nc.vector.tensor_scalar(
                        out=xn_c[:, t, :],
                        in0=xf[:, t, sl],
                        scalar1=sca[:, t, 0:1],
                        scalar2=sca[:, t, 1:2],
                        op0=OP.mult,
                        op1=OP.add,
                    )
                for oc in range(2):
                    ps = psA.tile([128, 512], F32, tag="mm")
                    for t in range(2):
                        nc.tensor.matmul(
                            out=ps[:],
                            lhsT=kwT_t[:, t, oc * 128 : (oc + 1) * 128],
                            rhs=xn_c[:, t, :],
                            start=(t == 0),
                            stop=(t == 1),
                        )
                    nc.vector.tensor_scalar(
                        out=k_t[:, oc, sl],
                        in0=ps[:],
                        scalar1=kb_t[:, oc, :],
                        scalar2=None,
                        op0=OP.add,
                    )
                for oc in range(2):
                    ps = psA.tile([128, 512], F32, tag="mm")
                    for t in range(2):
                        nc.tensor.matmul(
                            out=ps[:],
                            lhsT=vwT_t[:, t, oc * 128 : (oc + 1) * 128],
                            rhs=xn_c[:, t, :],
                            start=(t == 0),
                            stop=(t == 1),
                        )
                    nc.vector.tensor_copy(out=v_t[:, oc, sl], in_=ps[:])
            # Q projection (pre-scaled by 1/sqrt(C) on host): q[c, i]
            for ch in range(4):
                sl = slice(ch * 512, (ch + 1) * 512)
                xn_c = xnp.tile([128, 2, 512], F32R, tag="xn")
                for t in range(2):
                    nc.vector.tensor_scalar(
                        out=xn_c[:, t, :],
                        in0=xc[:, t, sl],
                        scalar1=sca[:, t, 0:1],
                        scalar2=sca[:, t, 1:2],
                        op0=OP.mult,
                        op1=OP.add,
                    )
                for oc in range(2):
                    ps = psA.tile([128, 512], F32, tag="mm")
                    for t in range(2):
                        nc.tensor.matmul(
                            out=ps[:],
                            lhsT=qwT_t[:, t, oc * 128 : (oc + 1) * 128],
                            rhs=xn_c[:, t, :],
                            start=(t == 0),
                            stop=(t == 1),
                        )
                    nc.vector.tensor_scalar(
                        out=q_t[:, oc, sl],
                        in0=ps[:],
                        scalar1=qb_t[:, oc, :],
                        scalar2=None,
                        op0=OP.add,
                    )
            # pvT[j, o] = (pw @ v)^T, with ones column at o=C
            nc.vector.memset(pvT[:, :, C : C + 1], 1.0)
            for jt in range(JT):
                ps = psA.tile([128, C], F32, tag="mm")
                for t in range(2):
                    nc.tensor.matmul(
                        out=ps[:],
                        lhsT=v_t[:, t, jt * 128 : (jt + 1) * 128],
                        rhs=pwT_t[:, t, :],
                        start=(t == 0),
                        stop=(t == 1),
                    )
                nc.vector.tensor_copy(out=pvT[:, jt, 0:C], in_=ps[:])

        # ---------------- Phase B: attention ------------------------------
        with (
            tc.tile_pool(name="attn", bufs=2) as attnp,
            tc.tile_pool(name="psS", bufs=6, space="PSUM") as psS,
            tc.tile_pool(name="psF", bufs=2, space="PSUM") as psF,
        ):
            for ib in range(NIB):
                isl_b = slice(ib * IBS, (ib + 1) * IBS)
                at = attnp.tile([128, JT, IBS], BF16, tag="attn")
                for jt in range(JT):
                    ps = psS.tile([128, IBS], F32, tag="s")
                    for t in range(2):
                        nc.tensor.matmul(
                            out=ps[:],
                            lhsT=k_t[:, t, jt * 128 : (jt + 1) * 128],
                            rhs=q_t[:, t, isl_b],
                            start=(t == 0),
                            stop=(t == 1),
                        )
                    nc.scalar.activation(
                        out=at[:, jt, :], in_=ps[:], func=AF.Exp, bias=0.0, scale=1.0
                    )
                for sl4 in range(IBS // 128):
                    isl = ib * 4 + sl4
                    pf = psF.tile([128, C + 1], F32, tag="fin")
                    for jt in range(JT):
                        nc.tensor.matmul(
                            out=pf[:],
                            lhsT=at[:, jt, sl4 * 128 : (sl4 + 1) * 128],
                            rhs=pvT[:, jt, :],
                            start=(jt == 0),
                            stop=(jt == JT - 1),
                        )
                    r = rpool.tile([128, 1], F32, tag="r")
                    nc.vector.reciprocal(out=r[:], in_=pf[:, C : C + 1])
                    o = outp.tile([128, C], F32, tag="o")
                    nc.vector.scalar_tensor_tensor(
                        out=o[:],
                        in0=pf[:, 0:C],
                        scalar=r[:],
                        in1=xT[:, isl, :],
                        op0=OP.mult,
                        op1=OP.add,
                    )
                    nc.sync.dma_start(
                        out=out_d[isl * 128 : (isl + 1) * 128, :], in_=o[:]
                    )

    split_sync_waits(nc)
    return nc


_CACHE = {}


def _get_program():
    if "nc" not in _CACHE:
        _CACHE["nc"] = _build_program()
    return _CACHE["nc"]


def kernel(x, gn_w, gn_b, qw, qb, kw, kb, vw, vb, pw, pb):
    x = np.asarray(x, dtype=np.float32)
    gn_w = np.asarray(gn_w, dtype=np.float32)
    gn_b = np.asarray(gn_b, dtype=np.float32)
    qw = np.asarray(qw, dtype=np.float32)
    qb = np.asarray(qb, dtype=np.float32)
    kw = np.asarray(kw, dtype=np.float32)
    kb = np.asarray(kb, dtype=np.float32)
    vw = np.asarray(vw, dtype=np.float32)
    vb = np.asarray(vb, dtype=np.float32)
    pw = np.asarray(pw, dtype=np.float32)
    pb = np.asarray(pb, dtype=np.float32)

    nc = _get_program()
    s = 1.0 / np.sqrt(C)
    qwT = np.ascontiguousarray((qw * s).T).astype(np.float32)
    kwT = np.ascontiguousarray(kw.T).astype(np.float32)
    vwT = np.ascontiguousarray(vw.T).astype(np.float32)
    pwT = np.ascontiguousarray(pw.T).astype(np.float32)
    qb2 = (qb * s).reshape(C, 1).astype(np.float32)
    kb2 = kb.reshape(C, 1).astype(np.float32)
    pbe = (pb + pw @ vb).reshape(1, C).astype(np.float32)
    gnw = gn_w.reshape(C, 1)
    gnb = gn_b.reshape(C, 1)
    ident = np.eye(128, dtype=np.float32)

    p_idx = np.arange(128)
    g_idx = np.arange(G)
    ind = np.zeros((128, 2 * G), dtype=np.float32)
    ind[:, :G] = (p_idx[:, None] // 8 == g_idx[None, :]).astype(np.float32)
    ind[:, G:] = (16 + p_idx[:, None] // 8 == g_idx[None, :]).astype(np.float32)
    bc = np.zeros((G, C), dtype=np.float32)
    bc[:, :128] = (g_idx[:, None] == p_idx[None, :] // 8).astype(np.float32)
    bc[:, 128:] = (g_idx[:, None] == 16 + p_idx[None, :] // 8).astype(np.float32)

    shared = {
        "qwT": qwT, "kwT": kwT, "vwT": vwT, "pwT": pwT,
        "qb2": qb2, "kb2": kb2, "pbe": pbe,
        "gnw": gnw, "gnb": gnb,
        "ident": ident, "ind": ind, "bc": bc,
    }
    in_maps = []
    for core in range(NCORES):
        b, h = core // 2, core % 2
        m = dict(shared)
        m["x_full"] = np.ascontiguousarray(x[b])
        m["x_cols"] = np.ascontiguousarray(x[b][:, h * LQ : (h + 1) * LQ])
        in_maps.append(m)

    res = run_bass_kernel_spmd(nc, in_maps, core_ids=list(range(NCORES)))

    out = np.empty((B, C, L), dtype=np.float32)
    for core in range(NCORES):
        b, h = core // 2, core % 2
        out[b, :, h * LQ : (h + 1) * LQ] = res.results[core]["out"].T
    return out


# revision 11
# speedup vs baseline: 1.1026x; 1.1026x over previous
"""AttnBlock (GroupNorm + single-head full attention + residual) on 8 TRN2 cores.

Reference computation (B=4, C=256, L=4096, fp32):
    xn   = GroupNorm32(x) * gn_w + gn_b
    q, k, v = 1x1 convs of xn;  attn = softmax(q^T k / sqrt(C)) ; out = x + pw @ (attn v)

Sharding: 8 cores = 4 batches x 2 query-halves. Each core computes GroupNorm +
K/V over the full sequence of its batch element, and Q/attention for its half
of the queries (Lq = 2048). No collectives; the host slices inputs and
concatenates outputs.

Per-core kernel structure:
  - GroupNorm stats via bn_stats/bn_aggr per partition row, then cross-partition
    group reduction + broadcast-back with tiny indicator matmuls on the PE.
  - Q/K/V projections with float32r matmuls (full PE rate, ~1e-4 accuracy).
  - v is immediately re-projected through pw:  pvT[j, o] = (pw @ v)^T, with an
    extra ones-column appended.  Then attention output and softmax row-sums
    come from ONE fused matmul chain:
        finT[i, (o|sum)] = sum_j exp(sT)[j, i] * pvT[j, (o|1)]
  - Scores are computed transposed (sT[j, i]) so softmax reduction over keys j
    is the matmul contraction, never a cross-partition op.  Logits are in
    [-6.2, 6.0] (std ~1 by construction), so exp needs no max subtraction.
  - Normalize + residual in one DVE op: out = (finT * 1/sum) + (x^T + pb_eff),
    with x^T built once via PE transposes.  Output is [Lq, C] per core; the
    host transposes back.  vb/pb fold into pb_eff = pb + pw @ vb on the host.
"""

import numpy as np
from contextlib import ExitStack

import concourse.bass as bass
import concourse.tile as tile
from concourse import mybir
from concourse.bass_utils import run_bass_kernel_spmd
from concourse.vector_clock import ScopedClock
import bass_rust

F32 = mybir.dt.float32
F32R = mybir.dt.float32r
BF16 = mybir.dt.bfloat16
AF = mybir.ActivationFunctionType
OP = mybir.AluOpType

B, C, L = 4, 256, 4096
G = 32
EPS = 1e-6
NCORES = 8
LQ = L // 2  # queries per core
CT = C // 128  # 2 channel tiles
JT = L // 128  # 32 key tiles
NIB = 4  # i-blocks of 512 queries
IBS = 512
NIS = LQ // 128  # 16 query slices of 128


class TC(tile.TileContext):
    """This walrus build caps sync-waits per instruction at 1; Tile attaches
    several to one instruction.  Hoist extras onto same-engine NOPs."""

    def _drain_and_barrier(self, tick_clock, wait_clock):
        collector = self.nc.sync.nop(nofuse=True)
        wait_clock.add_sem_waits(
            collector.ins, ScopedClock({None: tick_clock.global_clock})
        )
        waits = (
            list(collector.ins.sync_info.on_wait)
            if collector.ins.sync_info is not None
            else []
        )
        collector.ins.sync_info = bass_rust.SyncInfo(on_wait=[], on_update=[])
        for w in waits:
            n2 = self.nc.sync.nop(nofuse=True)
            n2.ins.sync_info = bass_rust.SyncInfo(on_wait=[w], on_update=[])
        self.nc.sync.drain()
        self.nc.all_engine_barrier()
        assert self.sems is not None
        popped = self.nc._tile_sem_poison_stack.pop()
        assert popped is self._sem_poison
        self.nc.clear_and_free_semaphores(list(self.sems.allocated().values()))
        self.nc.all_engine_barrier()


def split_sync_waits(nc, max_waits=1):
    ctr = 0
    for fn in nc.m.functions:
        for bb in fn.blocks:
            old = list(bb.instructions)
            new = []
            changed = False
            for inst in old:
                si = inst.sync_info
                if si is not None and len(si.on_wait) > max_waits:
                    waits = list(si.on_wait)
                    extra, keep = waits[:-max_waits], waits[-max_waits:]
                    for i in range(0, len(extra), max_waits):
                        nop = mybir.InstNoOp(name=f"I-waitnop-{ctr}")
                        ctr += 1
                        nop.engine = inst.engine
                        nop.sync_info = bass_rust.SyncInfo(
                            on_wait=extra[i : i + max_waits], on_update=[]
                        )
                        new.append(nop)
                        changed = True
                    inst.sync_info = bass_rust.SyncInfo(
                        on_wait=keep, on_update=list(si.on_update)
                    )
                new.append(inst)
            if changed:
                bb.instructions = new


def _build_program(ZERO_BIAS):
    nc = bass.Bass()

    x_d = nc.declare_dram_parameter("x_full", [C, L], F32, isOutput=False)
    xc_d = nc.declare_dram_parameter("x_cols", [C, LQ], F32, isOutput=False)
    qwT_d = nc.declare_dram_parameter("qwT", [C, C], F32, isOutput=False)
    kwT_d = nc.declare_dram_parameter("kwT", [C, C], F32, isOutput=False)
    pvwT_d = nc.declare_dram_parameter("pvwT", [C, C], F32, isOutput=False)
    qb_d = nc.declare_dram_parameter("qb2", [C, 1], F32, isOutput=False)
    kb_d = nc.declare_dram_parameter("kb2", [C, 1], F32, isOutput=False)
    pbe_d = nc.declare_dram_parameter("pbe", [1, C], F32, isOutput=False)
    gnw_d = nc.declare_dram_parameter("gnw", [C, 1], F32, isOutput=False)
    gnb_d = nc.declare_dram_parameter("gnb", [C, 1], F32, isOutput=False)
    id_d = nc.declare_dram_parameter("ident", [128, 128], F32, isOutput=False)
    ind_d = nc.declare_dram_parameter("ind", [128, 2 * G], F32, isOutput=False)
    bc_d = nc.declare_dram_parameter("bc", [G, C], F32, isOutput=False)
    out_d = nc.declare_dram_parameter("out", [LQ, C], F32, isOutput=True)

    with TC(nc) as tc, ExitStack() as ctx:
        const = ctx.enter_context(tc.tile_pool(name="const", bufs=1))

        ident = const.tile([128, 128], F32, tag="ident")
        nc.sync.dma_start(out=ident[:], in_=id_d[:])
        ind_t = const.tile([128, 2, G], F32, tag="ind")
        nc.sync.dma_start(out=ind_t[:], in_=ind_d[:].rearrange("p (t g) -> p t g", t=2))
        bc_t = const.tile([G, 2, 128], F32, tag="bc")
        nc.sync.dma_start(out=bc_t[:], in_=bc_d[:].rearrange("g (t p) -> g t p", t=2))
        pbb = const.tile([128, C], F32, tag="pbb")
        nc.sync.dma_start(out=pbb[:], in_=pbe_d[:].to_broadcast([128, C]))

        def col2(tag, src):
            t = const.tile([128, 2, 1], F32, tag=tag)
            nc.sync.dma_start(
                out=t[:], in_=src[:].rearrange("(t p) o -> p t o", p=128)
            )
            return t

        gnw_t = col2("gnw", gnw_d)
        gnb_t = col2("gnb", gnb_d)
        qb_t = col2("qb", qb_d)
        kb_t = col2("kb", kb_d)

        qwT_t = const.tile([128, 2, C], F32R, tag="qwT")
        kwT_t = const.tile([128, 2, C], F32R, tag="kwT")
        pvwT_t = const.tile([128, 2, C], F32R, tag="pvwT")
        with tc.tile_pool(name="wstage", bufs=2) as wst:
            for w_d, w_t in (
                (qwT_d, qwT_t),
                (kwT_d, kwT_t),
                (pvwT_d, pvwT_t),
            ):
                st = wst.tile([128, 2, C], F32, tag="wst")
                nc.sync.dma_start(
                    out=st[:], in_=w_d[:].rearrange("(t p) o -> p t o", p=128)
                )
                nc.vector.tensor_copy(out=w_t[:], in_=st[:])

        xt_p = ctx.enter_context(tc.tile_pool(name="xt", bufs=1))
        outp = ctx.enter_context(tc.tile_pool(name="outp", bufs=4))
        qkv = ctx.enter_context(tc.tile_pool(name="qkv", bufs=1))
        pvt_p = ctx.enter_context(tc.tile_pool(name="pvt", bufs=1))
        small = ctx.enter_context(tc.tile_pool(name="small", bufs=1))
        rpool = ctx.enter_context(tc.tile_pool(name="rpool", bufs=4))

        xT = xt_p.tile([128, NIS, C], F32, tag="xT")
        q_t = qkv.tile([128, 2, LQ], F32R, tag="q")
        k_t = qkv.tile([128, 2, L], F32R, tag="k")
        pvT = pvt_p.tile([128, JT, C + 1], BF16, tag="pvT")

        # ---------------- Phase A: GroupNorm, projections, pvT, xT ----------
        with (
            tc.tile_pool(name="xbuf", bufs=1) as xbuf,
            tc.tile_pool(name="xnp", bufs=3) as xnp,
            tc.tile_pool(name="psA", bufs=4, space="PSUM") as psA,
        ):
            xf = xbuf.tile([128, 2, L], F32, tag="xf")
            xc = xbuf.tile([128, 2, LQ], F32, tag="xc")
            for t in range(2):
                for h in range(2):
                    nc.sync.dma_start(
                        out=xc[:, t, h * 1024 : (h + 1) * 1024],
                        in_=xc_d[:].rearrange("(t p) l -> p t l", p=128)[
                            :, t, h * 1024 : (h + 1) * 1024
                        ],
                    )
            for ch in range(8):
                sl = slice(ch * 512, (ch + 1) * 512)
                for t in range(2):
                    nc.sync.dma_start(
                        out=xf[:, t, sl],
                        in_=x_d[:].rearrange("(t p) l -> p t l", p=128)[:, t, sl],
                    )

            # GroupNorm statistics
            stats = small.tile([128, 2, 8, 6], F32, tag="stats")
            mv = small.tile([128, 2, 2], F32, tag="mv")
            for s in range(8):
                for t in range(2):
                    xv = xf[:, t, :].rearrange("p (s f) -> p s f", f=512)
                    nc.vector.bn_stats(out=stats[:, t, s, :], in_=xv[:, s, :])
            for t in range(2):
                nc.vector.bn_aggr(out=mv[:, t, :], in_=stats[:, t, :, :])
                # var slot <- E[x^2] = m*m + var
                nc.vector.tensor_scalar(
                    out=mv[:, t, 1:2],
                    in0=mv[:, t, 0:1],
                    scalar1=mv[:, t, 0:1],
                    scalar2=mv[:, t, 1:2],
                    op0=OP.mult,
                    op1=OP.add,
                )
            psg = psA.tile([G, 2], F32, tag="mm")
            nc.tensor.matmul(
                out=psg[:], lhsT=ind_t[:, 0, :], rhs=mv[:, 0, :], start=True, stop=False
            )
            nc.tensor.matmul(
                out=psg[:], lhsT=ind_t[:, 1, :], rhs=mv[:, 1, :], start=False, stop=True
            )
            g2 = small.tile([G, 2], F32, tag="g2")  # [mu, rstd]
            nvar = small.tile([G, 1], F32, tag="nvar")
            sq = small.tile([G, 1], F32, tag="sq")
            eps_t = small.tile([G, 1], F32, tag="eps")
            nc.vector.memset(eps_t[:], float(EPS))
            nc.vector.tensor_scalar_mul(out=g2[:, 0:1], in0=psg[:, 0:1], scalar1=0.125)
            nc.vector.tensor_scalar_mul(out=g2[:, 1:2], in0=psg[:, 1:2], scalar1=0.125)
            nc.vector.tensor_scalar(
                out=nvar[:],
                in0=g2[:, 0:1],
                scalar1=g2[:, 0:1],
                scalar2=g2[:, 1:2],
                op0=OP.mult,
                op1=OP.subtract,
            )  # mu^2 - E[x^2] = -var
            nc.scalar.activation(
                out=sq[:], in_=nvar[:], func=AF.Sqrt, bias=eps_t[:], scale=-1.0
            )
            nc.vector.reciprocal(out=g2[:, 1:2], in_=sq[:])

            # broadcast group stats back to channels; per-channel scale/bias
            sca = small.tile([128, 2, 2], F32, tag="sca")  # [s, t] per channel tile
            mneg = small.tile([128, 1], F32, tag="mneg")
            for t in range(2):
                psb = psA.tile([128, 2], F32, tag="mm")
                nc.tensor.matmul(
                    out=psb[:], lhsT=bc_t[:, t, :], rhs=g2[:], start=True, stop=True
                )
                nc.vector.tensor_mul(
                    out=sca[:, t, 0:1], in0=psb[:, 1:2], in1=gnw_t[:, t, :]
                )
                nc.vector.tensor_scalar_mul(
                    out=mneg[:], in0=psb[:, 0:1], scalar1=-1.0
                )
                nc.vector.scalar_tensor_tensor(
                    out=sca[:, t, 1:2],
                    in0=mneg[:],
                    scalar=sca[:, t, 0:1],
                    in1=gnb_t[:, t, :],
                    op0=OP.mult,
                    op1=OP.add,
                )

            # x^T for the residual (+ pb_eff folded in), from original x_cols
            for isl in range(NIS):
                for t in range(2):
                    pst = psA.tile([128, 128], F32, tag="mm")
                    nc.tensor.transpose(
                        out=pst[:],
                        in_=xc[:, t, isl * 128 : (isl + 1) * 128],
                        identity=ident[:],
                    )
                    nc.vector.tensor_add(
                        out=xT[:, isl, t * 128 : (t + 1) * 128],
                        in0=pst[:],
                        in1=pbb[:, t * 128 : (t + 1) * 128],
                    )

            # GroupNorm apply (rounding to f32r) + K/pvT/Q projections, streamed
            # per 512-column chunk so the normalized activations never live in
            # full in SBUF.  psum->SBUF copies ride the otherwise-idle ScalarE
            # when the projection biases are zero (DVE adds them otherwise).
            nc.vector.memset(pvT[:, :, C : C + 1], 1.0)
            for ch in range(8):
                sl = slice(ch * 512, (ch + 1) * 512)
                xn_c = xnp.tile([128, 2, 512], F32R, tag="xn")
                nc.vector.tensor_scalar(
                    out=xn_c[:, 0, :],
                    in0=xf[:, 0, sl],
                    scalar1=sca[:, 0, 0:1],
                    scalar2=sca[:, 0, 1:2],
                    op0=OP.mult,
                    op1=OP.add,
                )
                nc.gpsimd.tensor_scalar(
                    out=xn_c[:, 1, :],
                    in0=xf[:, 1, sl],
                    scalar1=sca[:, 1, 0:1],
                    scalar2=sca[:, 1, 1:2],
                    op0=OP.mult,
                    op1=OP.add,
                )
                for oc in range(2):
                    ps = psA.tile([128, 512], F32, tag="mm")
                    for t in range(2):
                        nc.tensor.matmul(
                            out=ps[:],
                            lhsT=kwT_t[:, t, oc * 128 : (oc + 1) * 128],
                            rhs=xn_c[:, t, :],
                            start=(t == 0),
                            stop=(t == 1),
                        )
                    if ZERO_BIAS:
                        nc.scalar.activation(
                            out=k_t[:, oc, sl], in_=ps[:], func=AF.Copy,
                            bias=0.0, scale=1.0,
                        )
                    else:
                        nc.vector.tensor_scalar(
                            out=k_t[:, oc, sl],
                            in0=ps[:],
                            scalar1=kb_t[:, oc, :],
                            scalar2=None,
                            op0=OP.add,
                        )
                for jl in range(4):
                    jt = ch * 4 + jl
                    ps = psA.tile([128, C], F32, tag="mm")
                    for t in range(2):
                        nc.tensor.matmul(
                            out=ps[:],
                            lhsT=xn_c[:, t, jl * 128 : (jl + 1) * 128],
                            rhs=pvwT_t[:, t, :],
                            start=(t == 0),
                            stop=(t == 1),
                        )
                    if jl % 2 == 0:
                        nc.scalar.activation(
                            out=pvT[:, jt, 0:C], in_=ps[:], func=AF.Copy,
                            bias=0.0, scale=1.0,
                        )
                    else:
                        nc.vector.tensor_copy(out=pvT[:, jt, 0:C], in_=ps[:])
            # Q projection (pre-scaled by 1/sqrt(C) on host): q[c, i]
            for ch in range(4):
                sl = slice(ch * 512, (ch + 1) * 512)
                xn_c = xnp.tile([128, 2, 512], F32R, tag="xn")
                nc.vector.tensor_scalar(
                    out=xn_c[:, 0, :],
                    in0=xc[:, 0, sl],
                    scalar1=sca[:, 0, 0:1],
                    scalar2=sca[:, 0, 1:2],
                    op0=OP.mult,
                    op1=OP.add,
                )
                nc.gpsimd.tensor_scalar(
                    out=xn_c[:, 1, :],
                    in0=xc[:, 1, sl],
                    scalar1=sca[:, 1, 0:1],
                    scalar2=sca[:, 1, 1:2],
                    op0=OP.mult,
                    op1=OP.add,
                )
                for oc in range(2):
                    ps = psA.tile([128, 512], F32, tag="mm")
                    for t in range(2):
                        nc.tensor.matmul(
                            out=ps[:],
                            lhsT=qwT_t[:, t, oc * 128 : (oc + 1) * 128],
                            rhs=xn_c[:, t, :],
                            start=(t == 0),
                            stop=(t == 1),
                        )
                    if ZERO_BIAS:
                        nc.scalar.activation(
                            out=q_t[:, oc, sl], in_=ps[:], func=AF.Copy,
                            bias=0.0, scale=1.0,
                        )
                    else:
                        nc.vector.tensor_scalar(
                            out=q_t[:, oc, sl],
                            in0=ps[:],
                            scalar1=qb_t[:, oc, :],
                            scalar2=None,
                            op0=OP.add,
                        )

        # ---------------- Phase B: attention ------------------------------
        with (
            tc.tile_pool(name="attn", bufs=2) as attnp,
            tc.tile_pool(name="psS", bufs=6, space="PSUM") as psS,
            tc.tile_pool(name="psF", bufs=2, space="PSUM") as psF,
        ):
            for ib in range(NIB):
                isl_b = slice(ib * IBS, (ib + 1) * IBS)
                at = attnp.tile([128, JT, IBS], BF16, tag="attn")
                for jt in range(JT):
                    ps = psS.tile([128, IBS], F32, tag="s")
                    for t in range(2):
                        nc.tensor.matmul(
                            out=ps[:],
                            lhsT=k_t[:, t, jt * 128 : (jt + 1) * 128],
                            rhs=q_t[:, t, isl_b],
                            start=(t == 0),
                            stop=(t == 1),
                        )
                    nc.scalar.activation(
                        out=at[:, jt, :], in_=ps[:], func=AF.Exp, bias=0.0, scale=1.0
                    )
                for sl4 in range(IBS // 128):
                    isl = ib * 4 + sl4
                    pf = psF.tile([128, C + 1], F32, tag="fin")
                    for jt in range(JT):
                        nc.tensor.matmul(
                            out=pf[:],
                            lhsT=at[:, jt, sl4 * 128 : (sl4 + 1) * 128],
                            rhs=pvT[:, jt, :],
                            start=(jt == 0),
                            stop=(jt == JT - 1),
                        )
                    r = rpool.tile([128, 1], F32, tag="r")
                    nc.vector.reciprocal(out=r[:], in_=pf[:, C : C + 1])
                    o = outp.tile([128, C], F32, tag="o")
                    nc.vector.scalar_tensor_tensor(
                        out=o[:],
                        in0=pf[:, 0:C],
                        scalar=r[:],
                        in1=xT[:, isl, :],
                        op0=OP.mult,
                        op1=OP.add,
                    )
                    nc.sync.dma_start(
                        out=out_d[isl * 128 : (isl + 1) * 128, :], in_=o[:]
                    )

    split_sync_waits(nc)
    return nc


_CACHE = {}


def _get_program(zero_bias=True):
    key = ("nc", bool(zero_bias))
    if key not in _CACHE:
        _CACHE[key] = _build_program(bool(zero_bias))
    return _CACHE[key]


def kernel(x, gn_w, gn_b, qw, qb, kw, kb, vw, vb, pw, pb):
    x = np.asarray(x, dtype=np.float32)
    gn_w = np.asarray(gn_w, dtype=np.float32)
    gn_b = np.asarray(gn_b, dtype=np.float32)
    qw = np.asarray(qw, dtype=np.float32)
    qb = np.asarray(qb, dtype=np.float32)
    kw = np.asarray(kw, dtype=np.float32)
    kb = np.asarray(kb, dtype=np.float32)
    vw = np.asarray(vw, dtype=np.float32)
    vb = np.asarray(vb, dtype=np.float32)
    pw = np.asarray(pw, dtype=np.float32)
    pb = np.asarray(pb, dtype=np.float32)

    zero_bias = not (np.any(qb) or np.any(kb))
    nc = _get_program(zero_bias)
    s = 1.0 / np.sqrt(C)
    qwT = np.ascontiguousarray((qw * s).T).astype(np.float32)
    kwT = np.ascontiguousarray(kw.T).astype(np.float32)
    pvw = (pw.astype(np.float64) @ vw.astype(np.float64)).astype(np.float32)
    pvwT = np.ascontiguousarray(pvw.T)
    qb2 = (qb * s).reshape(C, 1).astype(np.float32)
    kb2 = kb.reshape(C, 1).astype(np.float32)
    pbe = (pb + pw @ vb).reshape(1, C).astype(np.float32)
    gnw = gn_w.reshape(C, 1)
    gnb = gn_b.reshape(C, 1)
    ident = np.eye(128, dtype=np.float32)

    p_idx = np.arange(128)
    g_idx = np.arange(G)
    ind = np.zeros((128, 2 * G), dtype=np.float32)
    ind[:, :G] = (p_idx[:, None] // 8 == g_idx[None, :]).astype(np.float32)
    ind[:, G:] = (16 + p_idx[:, None] // 8 == g_idx[None, :]).astype(np.float32)
    bc = np.zeros((G, C), dtype=np.float32)
    bc[:, :128] = (g_idx[:, None] == p_idx[None, :] // 8).astype(np.float32)
    bc[:, 128:] = (g_idx[:, None] == 16 + p_idx[None, :] // 8).astype(np.float32)

    shared = {
        "qwT": qwT, "kwT": kwT, "pvwT": pvwT,
        "qb2": qb2, "kb2": kb2, "pbe": pbe,
        "gnw": gnw, "gnb": gnb,
        "ident": ident, "ind": ind, "bc": bc,
    }
    in_maps = []
    for core in range(NCORES):
        b, h = core // 2, core % 2
        m = dict(shared)
        m["x_full"] = np.ascontiguousarray(x[b])
        m["x_cols"] = np.ascontiguousarray(x[b][:, h * LQ : (h + 1) * LQ])
        in_maps.append(m)

    res = run_bass_kernel_spmd(nc, in_maps, core_ids=list(range(NCORES)))

    out = np.empty((B, C, L), dtype=np.float32)
    for core in range(NCORES):
        b, h = core // 2, core % 2
        out[b, :, h * LQ : (h + 1) * LQ] = res.results[core]["out"].T
    return out
